# revision 10
# baseline (speedup 1.0000x reference)
"""AttentionBlock (GroupNorm32 + QKV 8-head attention + proj + residual) on 8 TRN2 NeuronCores.

Sharding: pure data-parallel over batch B=8 — one batch element per core.

v2 design (fp8 DoubleRow everywhere the error budget allows):
  - GroupNorm in f32 -> xn in bf16 (for q/k matmuls) and fp8 (for the v matmul).
  - q/k produced by bf16 matmuls, stored fp8 in a [32-partition, 2-subtile] slab
    layout so QK logits run as K=32x2 DoubleRow quadrant matmuls (2x col rate).
  - exp on the Scalar engine is the critical resource (~66us): it starts as soon
    as group-0's q/k slabs exist and must never starve.  QK psum tiles are
    [128, 2head, 512t] with bufs=2 so QK(sm+1) overlaps exp(sm).
  - AV is fp8 DoubleRow over sm-pairs, trailing the exp stream; softmax row-sums
    come from a ones-column in vT; per-(group, n) normalization uses
    reciprocal_approx_fast + a tiny select matmul broadcast.
  - proj is fp8 DoubleRow on the tail; residual path stays exact f32.
"""

import numpy as np
import ml_dtypes
from contextlib import ExitStack

import concourse.bass as bass
import concourse.tile as tile
from concourse import bacc, mybir
from concourse.bass_utils import run_bass_kernel_spmd

F32 = mybir.dt.float32
BF = mybir.dt.bfloat16
F8 = mybir.dt.float8e4
MULT = mybir.AluOpType.mult
ADD = mybir.AluOpType.add
AFT = mybir.ActivationFunctionType
DR = mybir.MatmulPerfMode.DoubleRow

C, T, H, CH = 512, 1024, 8, 64
NJ = C // 128          # 4 c-tiles
NTM = T // 128         # 8 t-tiles (s-chunks)
EXP_SCALE = float(CH) ** -0.5  # folded (q*s)·(k*s) scale, s = ch**-0.25
EXP_BIAS = -2.0                # keeps exp() under fp8e4 max (240); cancels in softmax

BF_NP = ml_dtypes.bfloat16
F8_NP = ml_dtypes.float8_e4m3


def build_graph(enable_asserts: bool = False):
    nc = bacc.Bacc(
        "TRN2",
        target_bir_lowering=False,
        debug=False,
        enable_asserts=enable_asserts,
    )
    x_d = nc.dram_tensor("x", [C, T], F32, kind="ExternalInput").ap()
    wq_d = nc.dram_tensor("wq", [C, C], BF, kind="ExternalInput").ap()   # cols slab-permuted
    wk_d = nc.dram_tensor("wk", [C, C], BF, kind="ExternalInput").ap()   # cols slab-permuted
    wv_d = nc.dram_tensor("wv", [C, C], F8, kind="ExternalInput").ap()   # natural head-major cols
    pw_d = nc.dram_tensor("pw", [C, C], F8, kind="ExternalInput").ap()
    bq_d = nc.dram_tensor("bq", [C], F32, kind="ExternalInput").ap()     # slab-permuted
    bk_d = nc.dram_tensor("bk", [C], F32, kind="ExternalInput").ap()     # slab-permuted
    bv_d = nc.dram_tensor("bv", [C], F32, kind="ExternalInput").ap()
    pb_d = nc.dram_tensor("pb", [C], F32, kind="ExternalInput").ap()
    gns_d = nc.dram_tensor("gns", [C], F32, kind="ExternalInput").ap()
    gnb_d = nc.dram_tensor("gnb", [C], F32, kind="ExternalInput").ap()
    g8_d = nc.dram_tensor("g8", [128, 8], F32, kind="ExternalInput").ap()
    gt8_d = nc.dram_tensor("gt8", [8, 128], F32, kind="ExternalInput").ap()
    sel4_d = nc.dram_tensor("sel4", [4, 2 * 128], BF, kind="ExternalInput").ap()
    out_d = nc.dram_tensor("out", [C, T], F32, kind="ExternalOutput").ap()

    with tile.TileContext(nc) as tc, ExitStack() as ctx:
        consts = ctx.enter_context(tc.tile_pool(name="consts", bufs=1))
        bigs = ctx.enter_context(tc.tile_pool(name="bigs", bufs=1))
        ewp = ctx.enter_context(tc.tile_pool(name="ewp", bufs=2))
        work = ctx.enter_context(tc.tile_pool(name="work", bufs=2))
        outp = ctx.enter_context(tc.tile_pool(name="outp", bufs=2))
        psum = ctx.enter_context(tc.tile_pool(name="psum", bufs=1, space="PSUM"))

        # ---- persistent sbuf tensors ----
        xt = bigs.tile([128, NJ, T], F32)           # raw x, kept for residual
        xn = bigs.tile([128, NJ, T], BF)            # groupnormed x (bf16, q/k path)
        xn8 = bigs.tile([128, NJ, T], F8)           # groupnormed x (fp8, v path)
        q_sb = bigs.tile([128, 2, 2, T], F8)        # q slabs [part, g, sub, t]
        k_sb = bigs.tile([128, 2, 2, T], F8)        # k slabs
        vT_sb = bigs.tile([128, NTM, H, 128], F8)   # v transposed + ones col, padded to 128
        a_un = bigs.tile([128, NJ, T], BF)          # unnormalized attention out
        a_n = bigs.tile([128, NJ, T], F8)           # normalized attention out

        # ---- input DMAs (ordered by first use) ----
        for j in range(NJ):
            nc.sync.dma_start(xt[:, j, :], x_d[j * 128:(j + 1) * 128, :])
        gns_sb = consts.tile([128, NJ], F32)
        gnb_sb = consts.tile([128, NJ], F32)
        nc.sync.dma_start(gns_sb[:], bass.AP(tensor=gns_d.tensor, offset=0, ap=[[1, 128], [128, NJ]]))
        nc.sync.dma_start(gnb_sb[:], bass.AP(tensor=gnb_d.tensor, offset=0, ap=[[1, 128], [128, NJ]]))
        g8_sb = consts.tile([128, 8], F32)
        gt8_sb = consts.tile([8, 128], F32)
        nc.sync.dma_start(g8_sb[:], g8_d[:])
        nc.sync.dma_start(gt8_sb[:], gt8_d[:])

        wq_sb = consts.tile([128, NJ, C], BF)
        wk_sb = consts.tile([128, NJ, C], BF)
        wv_sb = consts.tile([128, NJ, C], F8)
        pw_sb = consts.tile([128, NJ, C], F8)
        for j in range(NJ):
            nc.sync.dma_start(wk_sb[:, j, :], wk_d[j * 128:(j + 1) * 128, :])
            nc.sync.dma_start(wq_sb[:, j, :], wq_d[j * 128:(j + 1) * 128, :])
        bq_sb = consts.tile([128, NJ], F32)
        bk_sb = consts.tile([128, NJ], F32)
        pb_sb = consts.tile([128, NJ], F32)
        for j in range(NJ):
            nc.sync.dma_start(bq_sb[:, j:j + 1], bq_d[j * 128:(j + 1) * 128])
            nc.sync.dma_start(bk_sb[:, j:j + 1], bk_d[j * 128:(j + 1) * 128])
        for j in range(NJ):
            nc.sync.dma_start(wv_sb[:, j, :], wv_d[j * 128:(j + 1) * 128, :])
        bv_bc = consts.tile([128, C], F32)      # v bias broadcast to all partitions
        nc.sync.dma_start(bv_bc[:], bass.AP(tensor=bv_d.tensor, offset=0, ap=[[0, 128], [1, C]]))
        sel4_sb = consts.tile([4, 2, 128], BF)
        nc.sync.dma_start(sel4_sb[:], sel4_d[:].rearrange("p (j m) -> p j m", j=2))
        for j in range(NJ):
            nc.sync.dma_start(pw_sb[:, j, :], pw_d[j * 128:(j + 1) * 128, :])
            nc.sync.dma_start(pb_sb[:, j:j + 1], pb_d[j * 128:(j + 1) * 128])
        eps_sb = consts.tile([128, 1], F32)
        nc.vector.memset(eps_sb[:], 1e-5)
        # zero the vT pad columns once (Ldweights loads the full 128-col slab)
        nc.vector.memset(vT_sb[:, :, :, CH + 1:128], 0.0)
        nb_sb = consts.tile([128, 1], F32)
        nc.vector.memset(nb_sb[:], EXP_BIAS)

        # ---- GroupNorm: per-partition stats, group-reduce via tiny f32 matmuls ----
        stats_sb = consts.tile([128, 3 * NJ], F32)  # mean | var | mean^2 per c-tile
        for j in range(NJ):
            st6 = work.tile([128, 2, 6], F32, tag="st6")
            nc.vector.bn_stats(st6[:, 0, :], xt[:, j, 0:512])
            nc.vector.bn_stats(st6[:, 1, :], xt[:, j, 512:1024])
            nc.vector.bn_aggr(stats_sb[:, 3 * j:3 * j + 2], st6[:])
            nc.vector.tensor_mul(stats_sb[:, 3 * j + 2:3 * j + 3],
                                 stats_sb[:, 3 * j:3 * j + 1],
                                 stats_sb[:, 3 * j:3 * j + 1])
        ps_st = psum.tile([8, 3 * NJ], F32, tag="acc", bufs=4)
        nc.tensor.matmul(ps_st[:], g8_sb[:], stats_sb[:], start=True, stop=True)
        st_g = work.tile([8, 3 * NJ], F32, tag="stg")
        nc.vector.tensor_scalar(st_g[:], ps_st[:], 1.0 / 16.0, None, op0=MULT)
        stv = st_g[:].rearrange("p (j c) -> p j c", c=3)
        bcin = work.tile([8, 8], F32, tag="bcin")
        vv = work.tile([8, NJ], F32, tag="vv")
        nc.vector.tensor_add(vv[:], stv[:, :, 1], stv[:, :, 2])
        m2 = work.tile([8, NJ], F32, tag="m2")
        nc.vector.tensor_mul(m2[:], stv[:, :, 0], stv[:, :, 0])
        nc.vector.tensor_sub(vv[:], vv[:], m2[:])
        nc.scalar.activation(vv[:], vv[:], AFT.Sqrt, bias=eps_sb[0:8, :], scale=1.0)
        # warm the Exp activation table now (after Sqrt) so the attention exp
        # stream doesn't pay the table load on its first instruction
        warm = work.tile([1, 1], BF, tag="warm", bufs=1)
        nc.scalar.activation(warm[:], eps_sb[0:1, :], AFT.Exp, bias=eps_sb[0:1, :], scale=1.0)
        nc.vector.tensor_copy(bcin[:, 0:4], stv[:, :, 0])
        nc.vector.reciprocal(bcin[:, 4:8], vv[:])
        ps_pp = psum.tile([128, 8], F32, tag="acc", bufs=4)
        nc.tensor.matmul(ps_pp[:], gt8_sb[:], bcin[:], start=True, stop=True)
        ab = work.tile([128, 2 * NJ], F32, tag="ab")   # scale | shift per c-tile
        t4 = work.tile([128, NJ], F32, tag="t4")
        nc.vector.tensor_mul(ab[:, 0:NJ], ps_pp[:, 4:8], gns_sb[:])
        nc.vector.tensor_mul(t4[:], ps_pp[:, 0:4], ab[:, 0:NJ])
        nc.vector.tensor_sub(ab[:, NJ:2 * NJ], gnb_sb[:], t4[:])
        for j in range(NJ):
            nc.vector.tensor_scalar(xn[:, j, :], xt[:, j, :],
                                    ab[:, j:j + 1], ab[:, NJ + j:NJ + j + 1],
                                    op0=MULT, op1=ADD)
            nc.vector.tensor_copy(xn8[:, j, :], xn[:, j, :])

        # ---- q/k slabs for group g: slab s = 2g+sub holds, at partition 32h'+r,
        # channel (4g+h')*64 + 32*sub + r (host permutes wq/wk columns to match).
        def emit_qk_slabs(g):
            for sub in range(2):
                s = 2 * g + sub
                psk = psum.tile([128, T], F32, tag="psw", bufs=2)
                for j in range(NJ):
                    for n in range(2):
                        nc.tensor.matmul(psk[:, 512 * n:512 * (n + 1)],
                                         wk_sb[:, j, 128 * s:128 * (s + 1)],
                                         xn[:, j, 512 * n:512 * (n + 1)],
                                         start=(j == 0), stop=(j == NJ - 1))
                nc.vector.tensor_scalar(k_sb[:, g, sub, :], psk[:], bk_sb[:, s:s + 1], None, op0=ADD)
                psq = psum.tile([128, T], F32, tag="psw", bufs=2)
                for j in range(NJ):
                    for n in range(2):
                        nc.tensor.matmul(psq[:, 512 * n:512 * (n + 1)],
                                         wq_sb[:, j, 128 * s:128 * (s + 1)],
                                         xn[:, j, 512 * n:512 * (n + 1)],
                                         start=(j == 0), stop=(j == NJ - 1))
                nc.vector.tensor_scalar(q_sb[:, g, sub, :], psq[:], bq_sb[:, s:s + 1], None, op0=ADD)

        emit_qk_slabs(0)

        # ---- vT via fp8 DoubleRow: out[t, c] = xn8(lhsT [c,t]) x Wv([c, c_out]) ----
        def emit_v(tm):
            psv = psum.tile([128, 512], F32, tag="acc", bufs=4)
            for jp in range(2):
                nc.tensor.matmul(psv[:],
                                 xn8[:, 2 * jp:2 * jp + 2, 128 * tm:128 * (tm + 1)],
                                 wv_sb[:, 2 * jp:2 * jp + 2, :],
                                 start=(jp == 0), stop=(jp == 1), perf_mode=DR)
            nc.vector.tensor_add(vT_sb[:, tm, :, 0:CH],
                                 psv[:].rearrange("p (h c) -> p h c", h=H),
                                 bv_bc[:].rearrange("p (h c) -> p h c", h=H))
            nc.vector.memset(vT_sb[:, tm, :, CH:CH + 1], 1.0)

        for tm in range(NTM):
            emit_v(tm)

        # ---- attention sweeps: per (g, n): QK -> exp -> AV, with fillers ----
        # rs4/rc4/rcb per (g, n): 4 heads' softmax denominators for 512 t's.
        def attention_sweep(g, n, fillers):
            ew = ewp.tile([128, NTM, 4, 512], F8, tag="ew")
            psas = [psum.tile([128, 512], F32, tag="acc", bufs=4, name=f"psa{g}{n}{hh}")
                    for hh in range(4)]
            for sm in range(NTM):
                for u in range(2):
                    psw = psum.tile([128, 2, 512], F32, tag="psw", bufs=2)
                    for uu in range(2):
                        hh = 2 * u + uu
                        nc.tensor.matmul(psw[:, uu, :],
                                         k_sb[32 * hh:32 * (hh + 1), g, :, 128 * sm:128 * (sm + 1)],
                                         q_sb[32 * hh:32 * (hh + 1), g, :, 512 * n:512 * (n + 1)],
                                         start=True, stop=True, perf_mode=DR,
                                         tile_position=(32 * hh, 0))
                    nc.scalar.activation(ew[:, sm, 2 * u:2 * u + 2, :], psw[:],
                                         AFT.Exp, bias=nb_sb[:], scale=EXP_SCALE)
                if fillers and sm % 2 == 0:
                    fillers.pop(0)()
                if sm % 2 == 1:
                    smp = sm // 2
                    for hh in range(4):
                        nc.tensor.matmul(psas[hh][:],
                                         vT_sb[:, sm - 1:sm + 1, 4 * g + hh, :],
                                         ew[:, sm - 1:sm + 1, hh, :],
                                         start=(smp == 0), stop=(smp == NTM // 2 - 1),
                                         perf_mode=DR)
            # evacuate AV + row sums; normalize
            stg = work.tile([65, 4, 512], F32, tag="stg2")
            for hh in range(4):
                h = 4 * g + hh
                nc.vector.tensor_copy(a_un[64 * (h % 2):64 * (h % 2) + 64, h // 2, 512 * n:512 * (n + 1)],
                                      psas[hh][0:CH, :])
                nc.vector.tensor_copy(stg[64:65, hh, :], psas[hh][CH:CH + 1, :])
            rs4 = work.tile([4, 512], F32, tag="rs4")
            nc.sync.dma_start(rs4[:], stg[64:65, :, :])
            rc4 = work.tile([4, 512], F32, tag="rc4")
            nc.vector.reciprocal_approx_fast(rc4[:], rs4[:])
            rcb = work.tile([4, 512], BF, tag="rcb")
            nc.vector.tensor_copy(rcb[:], rc4[:])
            for jj in range(2):
                j = 2 * g + jj
                psb = psum.tile([128, 512], F32, tag="acc", bufs=4, name=f"psb{g}{n}{jj}")
                nc.tensor.matmul(psb[:], sel4_sb[:, jj, :], rcb[:], start=True, stop=True)
                nc.vector.tensor_mul(a_n[:, j, 512 * n:512 * (n + 1)],
                                     a_un[:, j, 512 * n:512 * (n + 1)], psb[:])

        # fillers: (g0,n0): nothing pending (v done up front); (g0,n1): q/k slabs g1.
        # Each chunk is self-contained (psum alloc + all matmuls + evac) so the
        # psw pool rotation can't deadlock around a half-written tile.
        def qk_slab_filler(g):
            chunks = []

            def make(s, part):
                def fn():
                    ps = psum.tile([128, T], F32, tag="psw", bufs=2,
                                   name=f"ps{part}{s}")
                    w_sb = wk_sb if part == "k" else wq_sb
                    for j in range(NJ):
                        for nn in range(2):
                            nc.tensor.matmul(ps[:, 512 * nn:512 * (nn + 1)],
                                             w_sb[:, j, 128 * s:128 * (s + 1)],
                                             xn[:, j, 512 * nn:512 * (nn + 1)],
                                             start=(j == 0), stop=(j == NJ - 1))
                    dst = k_sb if part == "k" else q_sb
                    b_sb = bk_sb if part == "k" else bq_sb
                    nc.vector.tensor_scalar(dst[:, g, s - 2 * g, :], ps[:],
                                            b_sb[:, s:s + 1], None, op0=ADD)
                return fn

            for sub in range(2):
                s = 2 * g + sub
                for part in ("k", "q"):
                    chunks.append(make(s, part))
            return chunks

        attention_sweep(0, 0, [])
        attention_sweep(0, 1, qk_slab_filler(1))
        attention_sweep(1, 0, [])
        attention_sweep(1, 1, [])

        # ---- proj (fp8 DoubleRow) + residual ----
        for m in range(NJ):
            psp = psum.tile([128, T], F32, tag="psw", bufs=2)
            for n in range(2):
                for jp in range(2):
                    nc.tensor.matmul(psp[:, 512 * n:512 * (n + 1)],
                                     pw_sb[:, 2 * jp:2 * jp + 2, 128 * m:128 * (m + 1)],
                                     a_n[:, 2 * jp:2 * jp + 2, 512 * n:512 * (n + 1)],
                                     start=(jp == 0), stop=(jp == 1), perf_mode=DR)
            osb = outp.tile([128, T], F32, tag="osb")
            nc.vector.scalar_tensor_tensor(osb[:], psp[:], pb_sb[:, m:m + 1], xt[:, m, :],
                                           op0=ADD, op1=ADD)
            nc.sync.dma_start(out_d[128 * m:128 * (m + 1), :], osb[:])

    nc.compile()
    return nc


_NC_CACHE = {}


def get_nc():
    if "nc" not in _NC_CACHE:
        _NC_CACHE["nc"] = build_graph()
    return _NC_CACHE["nc"]


def make_in_maps(x, norm_scale, norm_bias, qkv_w, qkv_b, proj_w, proj_b):
    x = np.asarray(x, dtype=np.float32)
    B = x.shape[0]
    qr = np.asarray(qkv_w, np.float32).reshape(H, 3, CH, C)
    wq_n = qr[:, 0].reshape(C, C)   # [out_channel(head-major), c]
    wk_n = qr[:, 1].reshape(C, C)
    wv_n = qr[:, 2].reshape(C, C)
    br = np.asarray(qkv_b, np.float32).reshape(H, 3, CH)
    bq_n = br[:, 0].reshape(C)
    bk_n = br[:, 1].reshape(C)
    bv_n = br[:, 2].reshape(C)
    # slab permutation: slab s (=2g+sub), partition p=32h'+r -> channel (4g+h')*64+32*sub+r
    perm = np.empty(C, np.int64)
    for s in range(4):
        g, sub = divmod(s, 2)
        p = np.arange(128)
        perm[128 * s + p] = (4 * g + p // 32) * 64 + 32 * sub + (p % 32)
    wq = np.ascontiguousarray(wq_n[perm].T).astype(BF_NP)      # [c, permuted out]
    wk = np.ascontiguousarray(wk_n[perm].T).astype(BF_NP)
    bq = np.ascontiguousarray(bq_n[perm])
    bk = np.ascontiguousarray(bk_n[perm])
    wv = np.ascontiguousarray(wv_n.T).astype(F8_NP)            # [c, out(head-major)]
    bv = np.ascontiguousarray(bv_n)
    pw = np.ascontiguousarray(np.asarray(proj_w, np.float32).T).astype(F8_NP)
    pb = np.ascontiguousarray(np.asarray(proj_b, np.float32))
    g8 = np.zeros((128, 8), np.float32)
    g8[np.arange(128), np.arange(128) // 16] = 1.0
    gt8 = np.ascontiguousarray(g8.T)
    # sel4[hh, jj, p] = 1 where head-local hh = 2*jj + p//64 covers c-partition p
    sel4 = np.zeros((4, 2, 128), np.float32)
    for jj in range(2):
        p = np.arange(128)
        sel4[2 * jj + p // 64, jj, p] = 1.0
    sel4 = np.ascontiguousarray(sel4.reshape(4, 256)).astype(BF_NP)
    shared = dict(wq=wq, wk=wk, wv=wv, pw=pw, bq=bq, bk=bk, bv=bv, pb=pb,
                  sel4=sel4,
                  gns=np.ascontiguousarray(np.asarray(norm_scale, np.float32)),
                  gnb=np.ascontiguousarray(np.asarray(norm_bias, np.float32)),
                  g8=g8, gt8=gt8)
    in_maps = []
    for i in range(B):
        m = dict(shared)
        m["x"] = np.ascontiguousarray(x[i].reshape(C, T))
        in_maps.append(m)
    return in_maps


def kernel(x, norm_scale, norm_bias, qkv_w, qkv_b, proj_w, proj_b):
    x = np.asarray(x, dtype=np.float32)
    B, Cc, Hh, Ww = x.shape
    nc = get_nc()
    in_maps = make_in_maps(x, norm_scale, norm_bias, qkv_w, qkv_b, proj_w, proj_b)
    res = run_bass_kernel_spmd(nc, in_maps, core_ids=list(range(B)))
    out = np.stack([res.results[i]["out"] for i in range(B)])
    return out.reshape(B, Cc, Hh, Ww).astype(np.float32)


# revision 16
# speedup vs baseline: 1.0873x; 1.0873x over previous
"""AttentionBlock (GroupNorm32 + QKV 8-head attention + proj + residual) on 8 TRN2 NeuronCores.

Sharding: pure data-parallel over batch B=8 — one batch element per core.

v3 design:
  - Scalar-engine exp of the 8 [1024,1024] logit tiles is the binding resource
    (~68us busy).  Everything else is scheduled around keeping it gap-free:
    the whole kernel is 4 "sweeps" (head-group g x t-half n) of
    QK -> exp -> AV, with all other work (v, spare q/k slabs, softmax
    normalization epilogues, first-half proj) run as fillers inside the
    exp windows.
  - QK: bf16 64-contraction quadrant matmuls (output-rate bound; fp8 DoubleRow
    does not raise the PE column rate, measured).
  - AV / v / proj: fp8 DoubleRow (2x contraction per column) with vT padded to
    128 weight columns (ISA requires full slabs); softmax row-sums via a
    ones-column in vT; denominators inverted with reciprocal_approx_fast and
    broadcast through a tiny f32r select matmul.
  - PSUM: tag "psw" [128,1024]x2 + tag "acc" [128,512]x4 = exactly 8 banks.
    All filler/epilogue chunks allocate an even number of psw-tag tiles so the
    double-buffer rotation parity stays aligned with the QK/exp pipeline.
  - Residual path stays exact f32.
"""

import numpy as np
import ml_dtypes
from contextlib import ExitStack

import concourse.bass as bass
import concourse.tile as tile
from concourse import bacc, mybir
from concourse.bass_utils import run_bass_kernel_spmd

F32 = mybir.dt.float32
F32R = mybir.dt.float32r
BF = mybir.dt.bfloat16
F8 = mybir.dt.float8e4
MULT = mybir.AluOpType.mult
ADD = mybir.AluOpType.add
AFT = mybir.ActivationFunctionType
DRM = mybir.MatmulPerfMode.DoubleRow

C, T, H, CH = 512, 1024, 8, 64
NJ = C // 128          # 4 c-tiles
NTM = T // 128         # 8 t-tiles (s-chunks)
EXP_SCALE = float(CH) ** -0.5  # folded (q*s)·(k*s) scale, s = ch**-0.25
EXP_BIAS = -2.0                # keeps exp() under fp8e4 max (240); cancels in softmax

BF_NP = ml_dtypes.bfloat16
F8_NP = ml_dtypes.float8_e4m3


def build_graph(enable_asserts: bool = False):
    nc = bacc.Bacc(
        "TRN2",
        target_bir_lowering=False,
        debug=False,
        enable_asserts=enable_asserts,
    )
    x_d = nc.dram_tensor("x", [C, T], F32, kind="ExternalInput").ap()
    wq_d = nc.dram_tensor("wq", [C, C], BF, kind="ExternalInput").ap()
    wk_d = nc.dram_tensor("wk", [C, C], BF, kind="ExternalInput").ap()
    wv_d = nc.dram_tensor("wv", [C, C], F8, kind="ExternalInput").ap()
    pw_d = nc.dram_tensor("pw", [C, C], F8, kind="ExternalInput").ap()
    bq_d = nc.dram_tensor("bq", [C], F32, kind="ExternalInput").ap()
    bk_d = nc.dram_tensor("bk", [C], F32, kind="ExternalInput").ap()
    bv_d = nc.dram_tensor("bv", [C], F32, kind="ExternalInput").ap()
    pb_d = nc.dram_tensor("pb", [C], F32, kind="ExternalInput").ap()
    gns_d = nc.dram_tensor("gns", [C], F32, kind="ExternalInput").ap()
    gnb_d = nc.dram_tensor("gnb", [C], F32, kind="ExternalInput").ap()
    g8_d = nc.dram_tensor("g8", [128, 8], F32, kind="ExternalInput").ap()   # pre-scaled 1/16
    gt8_d = nc.dram_tensor("gt8", [8, 128], F32, kind="ExternalInput").ap()
    sel4_d = nc.dram_tensor("sel4", [4, 2 * 128], BF, kind="ExternalInput").ap()
    out_d = nc.dram_tensor("out", [C, T], F32, kind="ExternalOutput").ap()

    with tile.TileContext(nc) as tc, ExitStack() as ctx:
        consts = ctx.enter_context(tc.tile_pool(name="consts", bufs=1))
        bigs = ctx.enter_context(tc.tile_pool(name="bigs", bufs=1))
        ewp = ctx.enter_context(tc.tile_pool(name="ewp", bufs=2))
        work = ctx.enter_context(tc.tile_pool(name="work", bufs=2))
        outp = ctx.enter_context(tc.tile_pool(name="outp", bufs=1))
        psum = ctx.enter_context(tc.tile_pool(name="psum", bufs=1, space="PSUM"))

        def pswt(name, shape=(128, 2, 512)):
            return psum.tile(list(shape), F32, tag="psw", bufs=2, name=name)

        def acct(name, shape=(128, 512)):
            return psum.tile(list(shape), F32, tag="acc", bufs=4, name=name)

        # ---- persistent sbuf tensors ----
        xt = bigs.tile([128, NJ, T], F32)           # raw x, kept for residual
        xn = bigs.tile([128, NJ, T], BF)            # groupnormed x (bf16, q/k path)
        xn8 = bigs.tile([128, NJ, T], F8)           # groupnormed x (fp8, v path)
        q_sb = bigs.tile([128, NJ, T], BF)          # q rows (head-major)
        k_sb = bigs.tile([128, NJ, T], BF)          # k rows (head-major)
        vT_sb = bigs.tile([128, NTM, H, 128], F8)   # v transposed + ones col, padded
        a_un = bigs.tile([128, NJ, T], BF)          # unnormalized attention out
        a_n = bigs.tile([128, NJ, T], F8)           # normalized attention out
        osb = outp.tile([128, NJ, T], F32)          # assembled output

        # ---- input DMAs (ordered by first use; x split fine for queue spread) ----
        for j in range(NJ):
            for hf in range(2):
                nc.sync.dma_start(xt[:, j, 512 * hf:512 * (hf + 1)],
                                  x_d[j * 128:(j + 1) * 128, 512 * hf:512 * (hf + 1)])
        gns_sb = consts.tile([128, NJ], F32)
        gnb_sb = consts.tile([128, NJ], F32)
        nc.sync.dma_start(gns_sb[:], bass.AP(tensor=gns_d.tensor, offset=0, ap=[[1, 128], [128, NJ]]))
        nc.sync.dma_start(gnb_sb[:], bass.AP(tensor=gnb_d.tensor, offset=0, ap=[[1, 128], [128, NJ]]))
        g8_sb = consts.tile([128, 8], F32)
        gt8_sb = consts.tile([8, 128], F32)
        nc.sync.dma_start(g8_sb[:], g8_d[:])
        nc.sync.dma_start(gt8_sb[:], gt8_d[:])

        wq_sb = consts.tile([128, NJ, C], BF)
        wk_sb = consts.tile([128, NJ, C], BF)
        wv_sb = consts.tile([128, NJ, C], F8)
        pw_sb = consts.tile([128, NJ, C], F8)
        for j in range(NJ):
            nc.sync.dma_start(wk_sb[:, j, :], wk_d[j * 128:(j + 1) * 128, :])
            nc.sync.dma_start(wq_sb[:, j, :], wq_d[j * 128:(j + 1) * 128, :])
        bq_sb = consts.tile([128, NJ], F32)
        bk_sb = consts.tile([128, NJ], F32)
        pb_sb = consts.tile([128, NJ], F32)
        for j in range(NJ):
            nc.sync.dma_start(bq_sb[:, j:j + 1], bq_d[j * 128:(j + 1) * 128])
            nc.sync.dma_start(bk_sb[:, j:j + 1], bk_d[j * 128:(j + 1) * 128])
        for j in range(NJ):
            nc.sync.dma_start(wv_sb[:, j, :], wv_d[j * 128:(j + 1) * 128, :])
        bv_bc = consts.tile([128, C], F32)      # v bias broadcast to all partitions
        nc.sync.dma_start(bv_bc[:], bass.AP(tensor=bv_d.tensor, offset=0, ap=[[0, 128], [1, C]]))
        sel4_sb = consts.tile([4, 2, 128], BF)
        nc.sync.dma_start(sel4_sb[:], sel4_d[:].rearrange("p (j m) -> p j m", j=2))
        for j in range(NJ):
            nc.sync.dma_start(pw_sb[:, j, :], pw_d[j * 128:(j + 1) * 128, :])
            nc.sync.dma_start(pb_sb[:, j:j + 1], pb_d[j * 128:(j + 1) * 128])

        eps_sb = consts.tile([128, 1], F32)
        nc.vector.memset(eps_sb[:], 1e-5)
        nb_sb = consts.tile([128, 1], F32)
        nc.vector.memset(nb_sb[:], EXP_BIAS)
        # zero the vT pad columns once (Ldweights loads the full 128-col slab)
        nc.vector.memset(vT_sb[:, :, :, CH + 1:128], 0.0)

        # ---- PE warm-up: ramp the tensor engine to full pstate during DMA wait
        wz = consts.tile([128, 512], BF)
        nc.vector.memset(wz[:], 0.0)
        wrd = consts.tile([128, 2], F32)
        for r in range(2):
            wps = pswt(f"wps{r}", (128, 2, 512))
            for i in range(8):
                nc.tensor.matmul(wps[:, i % 2, :], wz[:, 0:128], wz[:],
                                 start=True, stop=True)
            nc.vector.tensor_copy(wrd[:, r:r + 1], wps[:, 0, 0:1])

        # ---- GroupNorm: per-partition stats, group-reduce via tiny f32 matmuls ----
        stats_sb = consts.tile([128, 3, NJ], F32)  # rows: mean | var | mean^2
        for j in range(NJ):
            st6 = work.tile([128, 2, 6], F32, tag="st6")
            nc.vector.bn_stats(st6[:, 0, :], xt[:, j, 0:512])
            nc.vector.bn_stats(st6[:, 1, :], xt[:, j, 512:1024])
            nc.vector.bn_aggr(stats_sb[:, 0:2, j], st6[:])
        nc.vector.tensor_mul(stats_sb[:, 2, :], stats_sb[:, 0, :], stats_sb[:, 0, :])
        ps_st = acct("ps_st", (8, 3 * NJ))
        nc.tensor.matmul(ps_st[:], g8_sb[:], stats_sb[:].rearrange("p a b -> p (a b)"),
                         start=True, stop=True)
        st_g = work.tile([8, 3 * NJ], F32, tag="stg")
        nc.vector.tensor_copy(st_g[:], ps_st[:])
        stv = st_g[:].rearrange("p (c j) -> p c j", c=3)
        bcin = work.tile([8, 8], F32, tag="bcin")
        vv = work.tile([8, NJ], F32, tag="vv")
        nc.vector.tensor_add(vv[:], stv[:, 1, :], stv[:, 2, :])
        m2 = work.tile([8, NJ], F32, tag="m2")
        nc.vector.tensor_mul(m2[:], stv[:, 0, :], stv[:, 0, :])
        nc.vector.tensor_sub(vv[:], vv[:], m2[:])
        nc.vector.tensor_copy(bcin[:, 0:4], stv[:, 0, :])
        nc.scalar.activation(vv[:], vv[:], AFT.Sqrt, bias=eps_sb[0:8, :], scale=1.0)
        # re-warm the Exp table right after Sqrt so the stream isn't table-delayed
        warm = work.tile([1, 1], BF, tag="warm", bufs=1)
        nc.scalar.activation(warm[:], eps_sb[0:1, :], AFT.Exp, bias=eps_sb[0:1, :], scale=1.0)
        nc.vector.reciprocal(bcin[:, 4:8], vv[:])
        ps_pp = acct("ps_pp", (128, 8))
        nc.tensor.matmul(ps_pp[:], gt8_sb[:], bcin[:], start=True, stop=True)
        ab = work.tile([128, 2 * NJ], F32, tag="ab")   # scale | shift per c-tile
        t4 = work.tile([128, NJ], F32, tag="t4")
        nc.vector.tensor_mul(ab[:, 0:NJ], ps_pp[:, 4:8], gns_sb[:])
        nc.vector.tensor_mul(t4[:], ps_pp[:, 0:4], ab[:, 0:NJ])
        nc.vector.tensor_sub(ab[:, NJ:2 * NJ], gnb_sb[:], t4[:])
        for j in range(NJ):
            nc.vector.tensor_scalar(xn[:, j, :], xt[:, j, :],
                                    ab[:, j:j + 1], ab[:, NJ + j:NJ + j + 1],
                                    op0=MULT, op1=ADD)

        # ---- q/k for c-tiles (head pairs) m: bf16, LDWEIGHTS reused across n ----
        def emit_qk_ctile(m):
            psk = pswt(f"psk{m}", (128, T))
            for j in range(NJ):
                for n in range(2):
                    nc.tensor.matmul(psk[:, 512 * n:512 * (n + 1)],
                                     wk_sb[:, j, 128 * m:128 * (m + 1)],
                                     xn[:, j, 512 * n:512 * (n + 1)],
                                     start=(j == 0), stop=(j == NJ - 1))
            nc.vector.tensor_scalar(k_sb[:, m, :], psk[:], bk_sb[:, m:m + 1], None, op0=ADD)
            psq = pswt(f"psq{m}", (128, T))
            for j in range(NJ):
                for n in range(2):
                    nc.tensor.matmul(psq[:, 512 * n:512 * (n + 1)],
                                     wq_sb[:, j, 128 * m:128 * (m + 1)],
                                     xn[:, j, 512 * n:512 * (n + 1)],
                                     start=(j == 0), stop=(j == NJ - 1))
            nc.vector.tensor_scalar(q_sb[:, m, :], psq[:], bq_sb[:, m:m + 1], None, op0=ADD)

        emit_qk_ctile(0)
        emit_qk_ctile(1)
        # fp8 copy of xn for the v matmuls (off the q/k critical path)
        for j in range(NJ):
            nc.vector.tensor_copy(xn8[:, j, :], xn[:, j, :])

        # ---- filler chunk makers (each allocates an EVEN number of psw tiles) ----
        def v_pair(tm0):
            # two t-chunks of vT via fp8 DoubleRow
            def fn():
                for tm in (tm0, tm0 + 1):
                    psv = pswt(f"psv{tm}", (128, 512))
                    for jp in range(2):
                        nc.tensor.matmul(psv[:],
                                         xn8[:, 2 * jp:2 * jp + 2, 128 * tm:128 * (tm + 1)],
                                         wv_sb[:, 2 * jp:2 * jp + 2, :],
                                         start=(jp == 0), stop=(jp == 1), perf_mode=DRM)
                    nc.vector.tensor_add(vT_sb[:, tm, :, 0:CH],
                                         psv[:].rearrange("p (h c) -> p h c", h=H),
                                         bv_bc[:].rearrange("p (h c) -> p h c", h=H))
                    nc.vector.memset(vT_sb[:, tm, :, CH:CH + 1], 1.0)
            return fn

        def kq_ctile_chunk(m, part):
            # one c-tile of k or q as two [128,512] psum groups (n0, n1)
            def fn():
                w_sb = wk_sb if part == "k" else wq_sb
                dst = k_sb if part == "k" else q_sb
                b_sb = bk_sb if part == "k" else bq_sb
                for n in range(2):
                    ps = pswt(f"ps{part}{m}{n}", (128, 512))
                    for j in range(NJ):
                        nc.tensor.matmul(ps[:],
                                         w_sb[:, j, 128 * m:128 * (m + 1)],
                                         xn[:, j, 512 * n:512 * (n + 1)],
                                         start=(j == 0), stop=(j == NJ - 1))
                    nc.vector.tensor_scalar(dst[:, m, 512 * n:512 * (n + 1)], ps[:],
                                            b_sb[:, m:m + 1], None, op0=ADD)
            return fn

        def proj_half(ms, n):
            # proj output halves for c-tiles ms at t-half n (fp8 DoubleRow)
            def fn():
                for m in ms:
                    psp = pswt(f"psp{m}{n}", (128, 512))
                    for jp in range(2):
                        nc.tensor.matmul(psp[:],
                                         pw_sb[:, 2 * jp:2 * jp + 2, 128 * m:128 * (m + 1)],
                                         a_n[:, 2 * jp:2 * jp + 2, 512 * n:512 * (n + 1)],
                                         start=(jp == 0), stop=(jp == 1), perf_mode=DRM)
                    nc.vector.scalar_tensor_tensor(osb[:, m, 512 * n:512 * (n + 1)],
                                                   psp[:], pb_sb[:, m:m + 1],
                                                   xt[:, m, 512 * n:512 * (n + 1)],
                                                   op0=ADD, op1=ADD)
            return fn

        # ---- attention sweeps ----
        def attention_sweep(g, n, fillers):
            ew = ewp.tile([128, NTM, 4, 512], F8, tag="ew", name=f"ew{g}{n}")
            psas = [acct(f"psa{g}{n}{hh}") for hh in range(4)]
            for sm in range(NTM):
                for u in range(2):
                    p = 2 * g + u   # c-tile / head-pair index
                    psw = pswt(f"psw{g}{n}{sm}{u}")
                    for uu in range(2):
                        nc.tensor.matmul(psw[:, uu, :],
                                         k_sb[64 * uu:64 * (uu + 1), p, 128 * sm:128 * (sm + 1)],
                                         q_sb[64 * uu:64 * (uu + 1), p, 512 * n:512 * (n + 1)],
                                         start=True, stop=True, tile_position=(64 * uu, 0))
                    nc.scalar.activation(ew[:, sm, 2 * u:2 * u + 2, :], psw[:],
                                         AFT.Exp, bias=nb_sb[:], scale=EXP_SCALE)
                for f in fillers.get(sm, []):
                    f()
                if sm % 2 == 1 and sm < NTM - 1:
                    emit_av(g, n, ew, psas, sm // 2)
            return dict(g=g, n=n, ew=ew, psas=psas)

        def emit_av(g, n, ew, psas, smp):
            for hh in range(4):
                nc.tensor.matmul(psas[hh][:],
                                 vT_sb[:, 2 * smp:2 * smp + 2, 4 * g + hh, :],
                                 ew[:, 2 * smp:2 * smp + 2, hh, :],
                                 start=(smp == 0), stop=(smp == NTM // 2 - 1),
                                 perf_mode=DRM)

        # epilogue A: last AV pair, row-sum DMAs straight out of psum, reciprocal
        def ep_a(sw):
            g, n, ew, psas = sw["g"], sw["n"], sw["ew"], sw["psas"]
            stg = work.tile([65, 4, 512], F32, tag="stg")
            rs4 = work.tile([4, 512], F32, tag="rs4")
            rc4 = work.tile([4, 512], F32, tag="rc4")

            def fn():
                emit_av(g, n, ew, psas, NTM // 2 - 1)
                for hh in range(4):
                    nc.vector.tensor_copy(stg[64:65, hh, :], psas[hh][CH:CH + 1, :])
                nc.sync.dma_start(rs4[:], stg[64:65, :, :])
                for hh in range(4):
                    h = 4 * g + hh
                    nc.vector.tensor_copy(
                        a_un[64 * (h % 2):64 * (h % 2) + 64, h // 2, 512 * n:512 * (n + 1)],
                        psas[hh][0:CH, :])
                nc.vector.reciprocal_approx_fast(rc4[:], rs4[:])
            sw["rc4"] = rc4
            return fn

        # epilogue B: broadcast reciprocals (f32r matmul) and normalize a
        def ep_b(sw):
            g, n = sw["g"], sw["n"]

            def fn():
                rcb = work.tile([4, 512], BF, tag="rcb")
                nc.vector.tensor_copy(rcb[:], sw["rc4"][:])
                for jj in range(2):
                    j = 2 * g + jj
                    psb = pswt(f"psb{g}{n}{jj}", (128, 512))
                    nc.tensor.matmul(psb[:], sel4_sb[:, jj, :], rcb[:],
                                     start=True, stop=True)
                    nc.vector.tensor_mul(a_n[:, j, 512 * n:512 * (n + 1)],
                                         a_un[:, j, 512 * n:512 * (n + 1)], psb[:])
            return fn

        sw00 = attention_sweep(0, 0, {0: [v_pair(0)], 1: [v_pair(2)],
                                      2: [v_pair(4)], 3: [v_pair(6)]})
        sw01 = attention_sweep(0, 1, {0: [ep_a(sw00)],
                                      1: [kq_ctile_chunk(2, "k")],
                                      2: [kq_ctile_chunk(2, "q")],
                                      3: [kq_ctile_chunk(3, "k")],
                                      4: [ep_b(sw00)],
                                      5: [kq_ctile_chunk(3, "q")]})
        sw10 = attention_sweep(1, 0, {0: [ep_a(sw01)], 4: [ep_b(sw01)]})
        sw11 = attention_sweep(1, 1, {0: [ep_a(sw10)], 4: [ep_b(sw10)],
                                      5: [proj_half((0, 1), 0)],
                                      6: [proj_half((2, 3), 0)]})
        # tail
        ep_a(sw11)()
        ep_b(sw11)()
        proj_half((0, 1), 1)()
        proj_half((2, 3), 1)()
        for m in range(NJ):
            nc.sync.dma_start(out_d[128 * m:(m + 1) * 128, :], osb[:, m, :])

    nc.compile()
    return nc


_NC_CACHE = {}


def get_nc():
    if "nc" not in _NC_CACHE:
        _NC_CACHE["nc"] = build_graph()
    return _NC_CACHE["nc"]


def make_in_maps(x, norm_scale, norm_bias, qkv_w, qkv_b, proj_w, proj_b):
    x = np.asarray(x, dtype=np.float32)
    B = x.shape[0]
    qr = np.asarray(qkv_w, np.float32).reshape(H, 3, CH, C)
    wq = np.ascontiguousarray(qr[:, 0].reshape(C, C).T).astype(BF_NP)
    wk = np.ascontiguousarray(qr[:, 1].reshape(C, C).T).astype(BF_NP)
    wv = np.ascontiguousarray(qr[:, 2].reshape(C, C).T).astype(F8_NP)
    br = np.asarray(qkv_b, np.float32).reshape(H, 3, CH)
    bq = np.ascontiguousarray(br[:, 0].reshape(C))
    bk = np.ascontiguousarray(br[:, 1].reshape(C))
    bv = np.ascontiguousarray(br[:, 2].reshape(C))
    pw = np.ascontiguousarray(np.asarray(proj_w, np.float32).T).astype(F8_NP)
    pb = np.ascontiguousarray(np.asarray(proj_b, np.float32))
    g8 = np.zeros((128, 8), np.float32)
    g8[np.arange(128), np.arange(128) // 16] = 1.0 / 16.0
    gt8 = np.ascontiguousarray((g8 != 0).astype(np.float32).T)
    # sel4[hh, jj, p] = 1 where head-local hh = 2*jj + p//64 covers c-partition p
    sel4 = np.zeros((4, 2, 128), np.float32)
    for jj in range(2):
        p = np.arange(128)
        sel4[2 * jj + p // 64, jj, p] = 1.0
    sel4 = np.ascontiguousarray(sel4.reshape(4, 256)).astype(BF_NP)
    shared = dict(wq=wq, wk=wk, wv=wv, pw=pw, bq=bq, bk=bk, bv=bv, pb=pb,
                  sel4=sel4,
                  gns=np.ascontiguousarray(np.asarray(norm_scale, np.float32)),
                  gnb=np.ascontiguousarray(np.asarray(norm_bias, np.float32)),
                  g8=g8, gt8=gt8)
    in_maps = []
    for i in range(B):
        m = dict(shared)
        m["x"] = np.ascontiguousarray(x[i].reshape(C, T))
        in_maps.append(m)
    return in_maps


def kernel(x, norm_scale, norm_bias, qkv_w, qkv_b, proj_w, proj_b):
    x = np.asarray(x, dtype=np.float32)
    B, Cc, Hh, Ww = x.shape
    nc = get_nc()
    in_maps = make_in_maps(x, norm_scale, norm_bias, qkv_w, qkv_b, proj_w, proj_b)
    res = run_bass_kernel_spmd(nc, in_maps, core_ids=list(range(B)))
    out = np.stack([res.results[i]["out"] for i in range(B)])
    return out.reshape(B, Cc, Hh, Ww).astype(np.float32)


# revision 17
# speedup vs baseline: 1.1474x; 1.0553x over previous
"""AttentionBlock (GroupNorm32 + QKV 8-head attention + proj + residual) on 8 TRN2 NeuronCores.

Sharding: pure data-parallel over batch B=8 — one batch element per core.

v3 design:
  - Scalar-engine exp of the 8 [1024,1024] logit tiles is the binding resource
    (~68us busy).  Everything else is scheduled around keeping it gap-free:
    the whole kernel is 4 "sweeps" (head-group g x t-half n) of
    QK -> exp -> AV, with all other work (v, spare q/k slabs, softmax
    normalization epilogues, first-half proj) run as fillers inside the
    exp windows.
  - QK: bf16 64-contraction quadrant matmuls (output-rate bound; fp8 DoubleRow
    does not raise the PE column rate, measured).
  - AV / v / proj: fp8 DoubleRow (2x contraction per column) with vT padded to
    128 weight columns (ISA requires full slabs); softmax row-sums via a
    ones-column in vT; denominators inverted with reciprocal_approx_fast and
    broadcast through a tiny f32r select matmul.
  - PSUM: tag "psw" [128,1024]x2 + tag "acc" [128,512]x4 = exactly 8 banks.
    All filler/epilogue chunks allocate an even number of psw-tag tiles so the
    double-buffer rotation parity stays aligned with the QK/exp pipeline.
  - Residual path stays exact f32.
"""

import numpy as np
import ml_dtypes
from contextlib import ExitStack

import concourse.bass as bass
import concourse.tile as tile
from concourse import bacc, mybir
from concourse.bass_utils import run_bass_kernel_spmd

F32 = mybir.dt.float32
F32R = mybir.dt.float32r
BF = mybir.dt.bfloat16
F8 = mybir.dt.float8e4
MULT = mybir.AluOpType.mult
ADD = mybir.AluOpType.add
AFT = mybir.ActivationFunctionType
DRM = mybir.MatmulPerfMode.DoubleRow

C, T, H, CH = 512, 1024, 8, 64
NJ = C // 128          # 4 c-tiles
NTM = T // 128         # 8 t-tiles (s-chunks)
EXP_SCALE = float(CH) ** -0.5  # folded (q*s)·(k*s) scale, s = ch**-0.25
EXP_BIAS = -2.0                # keeps exp() under fp8e4 max (240); cancels in softmax

BF_NP = ml_dtypes.bfloat16
F8_NP = ml_dtypes.float8_e4m3


def build_graph(enable_asserts: bool = False):
    nc = bacc.Bacc(
        "TRN2",
        target_bir_lowering=False,
        debug=False,
        enable_asserts=enable_asserts,
    )
    x_d = nc.dram_tensor("x", [C, T], BF, kind="ExternalInput").ap()
    wq_d = nc.dram_tensor("wq", [C, C], BF, kind="ExternalInput").ap()
    wk_d = nc.dram_tensor("wk", [C, C], BF, kind="ExternalInput").ap()
    wv_d = nc.dram_tensor("wv", [C, C], F8, kind="ExternalInput").ap()
    pw_d = nc.dram_tensor("pw", [C, C], F8, kind="ExternalInput").ap()
    bq_d = nc.dram_tensor("bq", [C], F32, kind="ExternalInput").ap()
    bk_d = nc.dram_tensor("bk", [C], F32, kind="ExternalInput").ap()
    bv_d = nc.dram_tensor("bv", [C], F32, kind="ExternalInput").ap()
    pb_d = nc.dram_tensor("pb", [C], F32, kind="ExternalInput").ap()
    gns_d = nc.dram_tensor("gns", [C], F32, kind="ExternalInput").ap()
    gnb_d = nc.dram_tensor("gnb", [C], F32, kind="ExternalInput").ap()
    g8_d = nc.dram_tensor("g8", [128, 8], F32, kind="ExternalInput").ap()   # pre-scaled 1/16
    gt8_d = nc.dram_tensor("gt8", [8, 128], F32, kind="ExternalInput").ap()
    sel4_d = nc.dram_tensor("sel4", [4, 2 * 128], BF, kind="ExternalInput").ap()
    out_d = nc.dram_tensor("out", [C, T], F32, kind="ExternalOutput").ap()

    with tile.TileContext(nc) as tc, ExitStack() as ctx:
        consts = ctx.enter_context(tc.tile_pool(name="consts", bufs=1))
        bigs = ctx.enter_context(tc.tile_pool(name="bigs", bufs=1))
        ewp = ctx.enter_context(tc.tile_pool(name="ewp", bufs=2))
        work = ctx.enter_context(tc.tile_pool(name="work", bufs=2))
        outp = ctx.enter_context(tc.tile_pool(name="outp", bufs=1))
        psum = ctx.enter_context(tc.tile_pool(name="psum", bufs=1, space="PSUM"))

        def pswt(name, shape=(128, 2, 512)):
            return psum.tile(list(shape), F32, tag="psw", bufs=2, name=name)

        def acct(name, shape=(128, 512)):
            return psum.tile(list(shape), F32, tag="acc", bufs=4, name=name)

        # ---- persistent sbuf tensors ----
        xt = bigs.tile([128, NJ, T], BF)            # raw x (bf16), kept for residual
        xn = bigs.tile([128, NJ, T], BF)            # groupnormed x (bf16, q/k path)
        xn8 = bigs.tile([128, NJ, T], F8)           # groupnormed x (fp8, v path)
        q_sb = bigs.tile([128, NJ, T], BF)          # q rows (head-major)
        k_sb = bigs.tile([128, NJ, T], BF)          # k rows (head-major)
        vT_sb = bigs.tile([128, NTM, H, 128], F8)   # v transposed + ones col, padded
        a_un = bigs.tile([128, NJ, T], BF)          # unnormalized attention out
        a_n = bigs.tile([128, NJ, T], F8)           # normalized attention out
        osb = outp.tile([128, NJ, T], F32)          # assembled output

        # ---- input DMAs (ordered by first use; x split fine for queue spread) ----
        for j in range(NJ):
            for hf in range(2):
                nc.sync.dma_start(xt[:, j, 512 * hf:512 * (hf + 1)],
                                  x_d[j * 128:(j + 1) * 128, 512 * hf:512 * (hf + 1)])
        gns_sb = consts.tile([128, NJ], F32)
        gnb_sb = consts.tile([128, NJ], F32)
        nc.sync.dma_start(gns_sb[:], bass.AP(tensor=gns_d.tensor, offset=0, ap=[[1, 128], [128, NJ]]))
        nc.sync.dma_start(gnb_sb[:], bass.AP(tensor=gnb_d.tensor, offset=0, ap=[[1, 128], [128, NJ]]))
        g8_sb = consts.tile([128, 8], F32)
        gt8_sb = consts.tile([8, 128], F32)
        nc.sync.dma_start(g8_sb[:], g8_d[:])
        nc.sync.dma_start(gt8_sb[:], gt8_d[:])

        wq_sb = consts.tile([128, NJ, C], BF)
        wk_sb = consts.tile([128, NJ, C], BF)
        wv_sb = consts.tile([128, NJ, C], F8)
        pw_sb = consts.tile([128, NJ, C], F8)
        for j in range(NJ):
            nc.sync.dma_start(wk_sb[:, j, :], wk_d[j * 128:(j + 1) * 128, :])
            nc.sync.dma_start(wq_sb[:, j, :], wq_d[j * 128:(j + 1) * 128, :])
        bq_sb = consts.tile([128, NJ], F32)
        bk_sb = consts.tile([128, NJ], F32)
        pb_sb = consts.tile([128, NJ], F32)
        for j in range(NJ):
            nc.sync.dma_start(bq_sb[:, j:j + 1], bq_d[j * 128:(j + 1) * 128])
            nc.sync.dma_start(bk_sb[:, j:j + 1], bk_d[j * 128:(j + 1) * 128])
        for j in range(NJ):
            nc.sync.dma_start(wv_sb[:, j, :], wv_d[j * 128:(j + 1) * 128, :])
        bv_bc = consts.tile([128, C], F32)      # v bias broadcast to all partitions
        nc.sync.dma_start(bv_bc[:], bass.AP(tensor=bv_d.tensor, offset=0, ap=[[0, 128], [1, C]]))
        sel4_sb = consts.tile([4, 2, 128], BF)
        nc.sync.dma_start(sel4_sb[:], sel4_d[:].rearrange("p (j m) -> p j m", j=2))
        for j in range(NJ):
            nc.sync.dma_start(pw_sb[:, j, :], pw_d[j * 128:(j + 1) * 128, :])
            nc.sync.dma_start(pb_sb[:, j:j + 1], pb_d[j * 128:(j + 1) * 128])

        eps_sb = consts.tile([128, 1], F32)
        nc.vector.memset(eps_sb[:], 1e-5)
        nb_sb = consts.tile([128, 1], F32)
        nc.vector.memset(nb_sb[:], EXP_BIAS)
        # zero the vT pad columns once (Ldweights loads the full 128-col slab);
        # on GpSimd so it doesn't block the DVE preamble chain
        nc.gpsimd.memset(vT_sb[:, :, :, CH + 1:128], 0.0)

        # ---- PE warm-up: ramp the tensor engine to full pstate during DMA wait
        wz = consts.tile([128, 512], BF)
        nc.vector.memset(wz[:], 0.0)
        wrd = consts.tile([128, 2], F32)
        for r in range(2):
            wps = pswt(f"wps{r}", (128, 2, 512))
            for i in range(8):
                nc.tensor.matmul(wps[:, i % 2, :], wz[:, 0:128], wz[:],
                                 start=True, stop=True)
            nc.vector.tensor_copy(wrd[:, r:r + 1], wps[:, 0, 0:1])

        # ---- GroupNorm: per-partition stats, group-reduce via tiny f32 matmuls ----
        stats_sb = consts.tile([128, 3, NJ], F32)  # rows: mean | var | mean^2
        for j in range(NJ):
            st6 = work.tile([128, 2, 6], F32, tag="st6")
            nc.vector.bn_stats(st6[:, 0, :], xt[:, j, 0:512])
            nc.vector.bn_stats(st6[:, 1, :], xt[:, j, 512:1024])
            nc.vector.bn_aggr(stats_sb[:, 0:2, j], st6[:])
        nc.vector.tensor_mul(stats_sb[:, 2, :], stats_sb[:, 0, :], stats_sb[:, 0, :])
        ps_st = acct("ps_st", (8, 3 * NJ))
        nc.tensor.matmul(ps_st[:], g8_sb[:], stats_sb[:].rearrange("p a b -> p (a b)"),
                         start=True, stop=True)
        st_g = work.tile([8, 3 * NJ], F32, tag="stg")
        nc.vector.tensor_copy(st_g[:], ps_st[:])
        stv = st_g[:].rearrange("p (c j) -> p c j", c=3)
        bcin = work.tile([8, 8], F32, tag="bcin")
        vv = work.tile([8, NJ], F32, tag="vv")
        nc.vector.tensor_add(vv[:], stv[:, 1, :], stv[:, 2, :])
        m2 = work.tile([8, NJ], F32, tag="m2")
        nc.vector.tensor_mul(m2[:], stv[:, 0, :], stv[:, 0, :])
        nc.vector.tensor_sub(vv[:], vv[:], m2[:])
        nc.vector.tensor_copy(bcin[:, 0:4], stv[:, 0, :])
        nc.scalar.activation(vv[:], vv[:], AFT.Sqrt, bias=eps_sb[0:8, :], scale=1.0)
        # re-warm the Exp table right after Sqrt so the stream isn't table-delayed
        warm = work.tile([1, 1], BF, tag="warm", bufs=1)
        nc.scalar.activation(warm[:], eps_sb[0:1, :], AFT.Exp, bias=eps_sb[0:1, :], scale=1.0)
        nc.vector.reciprocal(bcin[:, 4:8], vv[:])
        ps_pp = acct("ps_pp", (128, 8))
        nc.tensor.matmul(ps_pp[:], gt8_sb[:], bcin[:], start=True, stop=True)
        ab = work.tile([128, 2 * NJ], F32, tag="ab")   # scale | shift per c-tile
        t4 = work.tile([128, NJ], F32, tag="t4")
        nc.vector.tensor_mul(ab[:, 0:NJ], ps_pp[:, 4:8], gns_sb[:])
        nc.vector.tensor_mul(t4[:], ps_pp[:, 0:4], ab[:, 0:NJ])
        nc.vector.tensor_sub(ab[:, NJ:2 * NJ], gnb_sb[:], t4[:])
        for j in range(NJ):
            nc.vector.tensor_scalar(xn[:, j, :], xt[:, j, :],
                                    ab[:, j:j + 1], ab[:, NJ + j:NJ + j + 1],
                                    op0=MULT, op1=ADD)

        # ---- q/k for c-tiles (head pairs) m: bf16, LDWEIGHTS reused across n ----
        def emit_qk_ctile(m):
            psk = pswt(f"psk{m}", (128, T))
            for j in range(NJ):
                for n in range(2):
                    nc.tensor.matmul(psk[:, 512 * n:512 * (n + 1)],
                                     wk_sb[:, j, 128 * m:128 * (m + 1)],
                                     xn[:, j, 512 * n:512 * (n + 1)],
                                     start=(j == 0), stop=(j == NJ - 1))
            nc.vector.tensor_scalar(k_sb[:, m, :], psk[:], bk_sb[:, m:m + 1], None, op0=ADD)
            psq = pswt(f"psq{m}", (128, T))
            for j in range(NJ):
                for n in range(2):
                    nc.tensor.matmul(psq[:, 512 * n:512 * (n + 1)],
                                     wq_sb[:, j, 128 * m:128 * (m + 1)],
                                     xn[:, j, 512 * n:512 * (n + 1)],
                                     start=(j == 0), stop=(j == NJ - 1))
            nc.vector.tensor_scalar(q_sb[:, m, :], psq[:], bq_sb[:, m:m + 1], None, op0=ADD)

        emit_qk_ctile(0)
        emit_qk_ctile(1)
        # fp8 copy of xn for the v matmuls (off the q/k critical path)
        for j in range(NJ):
            nc.vector.tensor_copy(xn8[:, j, :], xn[:, j, :])

        # ---- filler chunk makers (each allocates an EVEN number of psw tiles) ----
        def v_pair(tm0):
            # two t-chunks of vT via fp8 DoubleRow
            def fn():
                for tm in (tm0, tm0 + 1):
                    psv = pswt(f"psv{tm}", (128, 512))
                    for jp in range(2):
                        nc.tensor.matmul(psv[:],
                                         xn8[:, 2 * jp:2 * jp + 2, 128 * tm:128 * (tm + 1)],
                                         wv_sb[:, 2 * jp:2 * jp + 2, :],
                                         start=(jp == 0), stop=(jp == 1), perf_mode=DRM)
                    nc.vector.tensor_add(vT_sb[:, tm, :, 0:CH],
                                         psv[:].rearrange("p (h c) -> p h c", h=H),
                                         bv_bc[:].rearrange("p (h c) -> p h c", h=H))
                    nc.vector.memset(vT_sb[:, tm, :, CH:CH + 1], 1.0)
            return fn

        def kq_ctile_chunk(m, part):
            # one c-tile of k or q as two [128,512] psum groups (n0, n1)
            def fn():
                w_sb = wk_sb if part == "k" else wq_sb
                dst = k_sb if part == "k" else q_sb
                b_sb = bk_sb if part == "k" else bq_sb
                for n in range(2):
                    ps = pswt(f"ps{part}{m}{n}", (128, 512))
                    for j in range(NJ):
                        nc.tensor.matmul(ps[:],
                                         w_sb[:, j, 128 * m:128 * (m + 1)],
                                         xn[:, j, 512 * n:512 * (n + 1)],
                                         start=(j == 0), stop=(j == NJ - 1))
                    nc.vector.tensor_scalar(dst[:, m, 512 * n:512 * (n + 1)], ps[:],
                                            b_sb[:, m:m + 1], None, op0=ADD)
            return fn

        def proj_half(ms, n):
            # proj output halves for c-tiles ms at t-half n (fp8 DoubleRow)
            def fn():
                for m in ms:
                    psp = pswt(f"psp{m}{n}", (128, 512))
                    for jp in range(2):
                        nc.tensor.matmul(psp[:],
                                         pw_sb[:, 2 * jp:2 * jp + 2, 128 * m:128 * (m + 1)],
                                         a_n[:, 2 * jp:2 * jp + 2, 512 * n:512 * (n + 1)],
                                         start=(jp == 0), stop=(jp == 1), perf_mode=DRM)
                    nc.vector.scalar_tensor_tensor(osb[:, m, 512 * n:512 * (n + 1)],
                                                   psp[:], pb_sb[:, m:m + 1],
                                                   xt[:, m, 512 * n:512 * (n + 1)],
                                                   op0=ADD, op1=ADD)
            return fn

        # ---- attention sweeps ----
        def attention_sweep(g, n, fillers):
            ew = ewp.tile([128, NTM, 4, 512], F8, tag="ew", name=f"ew{g}{n}")
            psas = [acct(f"psa{g}{n}{hh}") for hh in range(4)]
            for sm in range(NTM):
                for u in range(2):
                    p = 2 * g + u   # c-tile / head-pair index
                    psw = pswt(f"psw{g}{n}{sm}{u}")
                    for uu in range(2):
                        nc.tensor.matmul(psw[:, uu, :],
                                         k_sb[64 * uu:64 * (uu + 1), p, 128 * sm:128 * (sm + 1)],
                                         q_sb[64 * uu:64 * (uu + 1), p, 512 * n:512 * (n + 1)],
                                         start=True, stop=True, tile_position=(64 * uu, 0))
                    nc.scalar.activation(ew[:, sm, 2 * u:2 * u + 2, :], psw[:],
                                         AFT.Exp, bias=nb_sb[:], scale=EXP_SCALE)
                if sm % 2 == 0 and sm >= 2:
                    emit_av(g, n, ew, psas, (sm - 2) // 2)
                for f in fillers.get(sm, []):
                    f()
            return dict(g=g, n=n, ew=ew, psas=psas)

        def emit_av(g, n, ew, psas, smp):
            for hh in range(4):
                nc.tensor.matmul(psas[hh][:],
                                 vT_sb[:, 2 * smp:2 * smp + 2, 4 * g + hh, :],
                                 ew[:, 2 * smp:2 * smp + 2, hh, :],
                                 start=(smp == 0), stop=(smp == NTM // 2 - 1),
                                 perf_mode=DRM)

        # epilogue A: last AV pair, row-sum DMAs straight out of psum, reciprocal
        def ep_a(sw):
            g, n, ew, psas = sw["g"], sw["n"], sw["ew"], sw["psas"]
            stg = work.tile([65, 4, 512], F32, tag="stg")
            rs4 = work.tile([4, 512], F32, tag="rs4")
            rc4 = work.tile([4, 512], F32, tag="rc4")

            def fn():
                emit_av(g, n, ew, psas, NTM // 2 - 1)
                for hh in range(4):
                    nc.vector.tensor_copy(stg[64:65, hh, :], psas[hh][CH:CH + 1, :])
                nc.sync.dma_start(rs4[:], stg[64:65, :, :])
                for hh in range(4):
                    h = 4 * g + hh
                    nc.vector.tensor_copy(
                        a_un[64 * (h % 2):64 * (h % 2) + 64, h // 2, 512 * n:512 * (n + 1)],
                        psas[hh][0:CH, :])
                nc.vector.reciprocal_approx_fast(rc4[:], rs4[:])
            sw["rc4"] = rc4
            return fn

        # epilogue B: broadcast reciprocals (f32r matmul) and normalize a
        def ep_b(sw):
            g, n = sw["g"], sw["n"]

            def fn():
                rcb = work.tile([4, 512], BF, tag="rcb")
                nc.vector.tensor_copy(rcb[:], sw["rc4"][:])
                for jj in range(2):
                    j = 2 * g + jj
                    psb = pswt(f"psb{g}{n}{jj}", (128, 512))
                    nc.tensor.matmul(psb[:], sel4_sb[:, jj, :], rcb[:],
                                     start=True, stop=True)
                    nc.vector.tensor_mul(a_n[:, j, 512 * n:512 * (n + 1)],
                                         a_un[:, j, 512 * n:512 * (n + 1)], psb[:])
            return fn

        sw00 = attention_sweep(0, 0, {0: [v_pair(0)], 2: [v_pair(2)],
                                      4: [v_pair(4)], 6: [v_pair(6)],
                                      3: [kq_ctile_chunk(2, "k")],
                                      7: [kq_ctile_chunk(2, "q")]})
        sw01 = attention_sweep(0, 1, {0: [ep_a(sw00)],
                                      1: [kq_ctile_chunk(3, "k")],
                                      3: [kq_ctile_chunk(3, "q")],
                                      4: [ep_b(sw00)]})
        sw10 = attention_sweep(1, 0, {0: [ep_a(sw01)], 4: [ep_b(sw01)]})
        sw11 = attention_sweep(1, 1, {0: [ep_a(sw10)], 4: [ep_b(sw10)],
                                      5: [proj_half((0, 1), 0)],
                                      6: [proj_half((2, 3), 0)]})
        # tail
        ep_a(sw11)()
        ep_b(sw11)()
        proj_half((0, 1), 1)()
        proj_half((2, 3), 1)()
        for m in range(NJ):
            nc.sync.dma_start(out_d[128 * m:(m + 1) * 128, :], osb[:, m, :])

    nc.compile()
    return nc


_NC_CACHE = {}


def get_nc():
    if "nc" not in _NC_CACHE:
        _NC_CACHE["nc"] = build_graph()
    return _NC_CACHE["nc"]


def make_in_maps(x, norm_scale, norm_bias, qkv_w, qkv_b, proj_w, proj_b):
    x = np.asarray(x, dtype=np.float32)
    B = x.shape[0]
    qr = np.asarray(qkv_w, np.float32).reshape(H, 3, CH, C)
    wq = np.ascontiguousarray(qr[:, 0].reshape(C, C).T).astype(BF_NP)
    wk = np.ascontiguousarray(qr[:, 1].reshape(C, C).T).astype(BF_NP)
    wv = np.ascontiguousarray(qr[:, 2].reshape(C, C).T).astype(F8_NP)
    br = np.asarray(qkv_b, np.float32).reshape(H, 3, CH)
    bq = np.ascontiguousarray(br[:, 0].reshape(C))
    bk = np.ascontiguousarray(br[:, 1].reshape(C))
    bv = np.ascontiguousarray(br[:, 2].reshape(C))
    pw = np.ascontiguousarray(np.asarray(proj_w, np.float32).T).astype(F8_NP)
    pb = np.ascontiguousarray(np.asarray(proj_b, np.float32))
    g8 = np.zeros((128, 8), np.float32)
    g8[np.arange(128), np.arange(128) // 16] = 1.0 / 16.0
    gt8 = np.ascontiguousarray((g8 != 0).astype(np.float32).T)
    # sel4[hh, jj, p] = 1 where head-local hh = 2*jj + p//64 covers c-partition p
    sel4 = np.zeros((4, 2, 128), np.float32)
    for jj in range(2):
        p = np.arange(128)
        sel4[2 * jj + p // 64, jj, p] = 1.0
    sel4 = np.ascontiguousarray(sel4.reshape(4, 256)).astype(BF_NP)
    shared = dict(wq=wq, wk=wk, wv=wv, pw=pw, bq=bq, bk=bk, bv=bv, pb=pb,
                  sel4=sel4,
                  gns=np.ascontiguousarray(np.asarray(norm_scale, np.float32)),
                  gnb=np.ascontiguousarray(np.asarray(norm_bias, np.float32)),
                  g8=g8, gt8=gt8)
    in_maps = []
    for i in range(B):
        m = dict(shared)
        m["x"] = np.ascontiguousarray(x[i].reshape(C, T)).astype(BF_NP)
        in_maps.append(m)
    return in_maps


def kernel(x, norm_scale, norm_bias, qkv_w, qkv_b, proj_w, proj_b):
    x = np.asarray(x, dtype=np.float32)
    B, Cc, Hh, Ww = x.shape
    nc = get_nc()
    in_maps = make_in_maps(x, norm_scale, norm_bias, qkv_w, qkv_b, proj_w, proj_b)
    res = run_bass_kernel_spmd(nc, in_maps, core_ids=list(range(B)))
    out = np.stack([res.results[i]["out"] for i in range(B)])
    return out.reshape(B, Cc, Hh, Ww).astype(np.float32)


# revision 19
# speedup vs baseline: 1.2277x; 1.0700x over previous
"""AttentionBlock (GroupNorm32 + QKV 8-head attention + proj + residual) on 8 TRN2 NeuronCores.

Sharding: pure data-parallel over batch B=8 — one batch element per core.

v5 design:
  - Scalar-engine exp of the logits is the binding resource (~66us busy).  The
    kernel is 4 sweeps (head-group g x t-half n) of 16 (pair, sm) units, each
    unit = 2 quadrant QK matmuls + one [128,1024] exp.  The exp stream must be
    gap-free: ONLY QK/exp use the "psw" psum tag (bufs=2); every other matmul
    (v, spare q/k c-tiles, AV bursts, recip broadcast, proj) runs as a
    self-contained chunk on the "acc" tag so the psw rotation never blocks.
  - AV is deferred: per (g,n,head) a 4-instruction fp8-DoubleRow burst over the
    completed ew tile, run as fillers in the NEXT sweep (last sweep: smp0-2
    right after its last QK, smp3 on the tail).  vT carries a ones-column for
    softmax row sums; denominators via reciprocal_approx_fast + select matmul.
  - QK is bf16 (output-rate bound; fp8 DoubleRow does not raise the column
    rate).  x input is bf16 (host-cast).  v/proj are fp8 DoubleRow.
"""

import numpy as np
import ml_dtypes
from contextlib import ExitStack

import concourse.bass as bass
import concourse.tile as tile
from concourse import bacc, mybir
from concourse.bass_utils import run_bass_kernel_spmd

F32 = mybir.dt.float32
BF = mybir.dt.bfloat16
F8 = mybir.dt.float8e4
MULT = mybir.AluOpType.mult
ADD = mybir.AluOpType.add
AFT = mybir.ActivationFunctionType
DRM = mybir.MatmulPerfMode.DoubleRow

C, T, H, CH = 512, 1024, 8, 64
NJ = C // 128          # 4 c-tiles
NTM = T // 128         # 8 t-tiles (s-chunks)
EXP_SCALE = float(CH) ** -0.5
EXP_BIAS = -2.0        # keeps exp() under fp8e4 max (240); cancels in softmax

BF_NP = ml_dtypes.bfloat16
F8_NP = ml_dtypes.float8_e4m3


def build_graph(enable_asserts: bool = False):
    nc = bacc.Bacc(
        "TRN2",
        target_bir_lowering=False,
        debug=False,
        enable_asserts=enable_asserts,
    )
    x_d = nc.dram_tensor("x", [C, T], BF, kind="ExternalInput").ap()
    wq_d = nc.dram_tensor("wq", [C, C], BF, kind="ExternalInput").ap()
    wk_d = nc.dram_tensor("wk", [C, C], BF, kind="ExternalInput").ap()
    wv_d = nc.dram_tensor("wv", [C, C], F8, kind="ExternalInput").ap()
    pw_d = nc.dram_tensor("pw", [C, C], F8, kind="ExternalInput").ap()
    bq_d = nc.dram_tensor("bq", [C], F32, kind="ExternalInput").ap()
    bk_d = nc.dram_tensor("bk", [C], F32, kind="ExternalInput").ap()
    bv_d = nc.dram_tensor("bv", [C], F32, kind="ExternalInput").ap()
    pb_d = nc.dram_tensor("pb", [C], F32, kind="ExternalInput").ap()
    gns_d = nc.dram_tensor("gns", [C], F32, kind="ExternalInput").ap()
    gnb_d = nc.dram_tensor("gnb", [C], F32, kind="ExternalInput").ap()
    g8_d = nc.dram_tensor("g8", [128, 8], F32, kind="ExternalInput").ap()   # pre-scaled 1/16
    gt8_d = nc.dram_tensor("gt8", [8, 128], F32, kind="ExternalInput").ap()
    sel4_d = nc.dram_tensor("sel4", [4, 2 * 128], BF, kind="ExternalInput").ap()
    out_d = nc.dram_tensor("out", [C, T], F32, kind="ExternalOutput").ap()

    with tile.TileContext(nc) as tc, ExitStack() as ctx:
        consts = ctx.enter_context(tc.tile_pool(name="consts", bufs=1))
        bigs = ctx.enter_context(tc.tile_pool(name="bigs", bufs=1))
        ewp = ctx.enter_context(tc.tile_pool(name="ewp", bufs=2))
        work = ctx.enter_context(tc.tile_pool(name="work", bufs=2))
        outp = ctx.enter_context(tc.tile_pool(name="outp", bufs=1))
        psum = ctx.enter_context(tc.tile_pool(name="psum", bufs=1, space="PSUM"))

        def pswt(name):
            return psum.tile([128, 2, 512], F32, tag="psw", bufs=2, name=name)

        def acct(name, shape=(128, 512)):
            return psum.tile(list(shape), F32, tag="acc", bufs=4, name=name)

        # ---- persistent sbuf tensors ----
        xt = bigs.tile([128, NJ, T], BF)            # raw x (bf16), kept for residual
        xn = bigs.tile([128, NJ, T], BF)            # groupnormed x (bf16, q/k path)
        xn8 = bigs.tile([128, NJ, T], F8)           # groupnormed x (fp8, v path)
        q_sb = bigs.tile([128, NJ, T], BF)          # q rows (head-major)
        k_sb = bigs.tile([128, NJ, T], BF)          # k rows (head-major)
        vT_sb = bigs.tile([128, NTM, H, 128], F8)   # v transposed + ones col, padded
        a_un = bigs.tile([128, NJ, T], BF)          # unnormalized attention out
        a_n = bigs.tile([128, NJ, T], F8)           # normalized attention out
        osb = outp.tile([128, NJ, T], F32)          # assembled output

        # ---- input DMAs (ordered by first use; x split fine for queue spread) ----
        for j in range(NJ):
            for hf in range(2):
                nc.sync.dma_start(xt[:, j, 512 * hf:512 * (hf + 1)],
                                  x_d[j * 128:(j + 1) * 128, 512 * hf:512 * (hf + 1)])
        gns_sb = consts.tile([128, NJ], F32)
        gnb_sb = consts.tile([128, NJ], F32)
        nc.sync.dma_start(gns_sb[:], bass.AP(tensor=gns_d.tensor, offset=0, ap=[[1, 128], [128, NJ]]))
        nc.sync.dma_start(gnb_sb[:], bass.AP(tensor=gnb_d.tensor, offset=0, ap=[[1, 128], [128, NJ]]))
        g8_sb = consts.tile([128, 8], F32)
        gt8_sb = consts.tile([8, 128], F32)
        nc.sync.dma_start(g8_sb[:], g8_d[:])
        nc.sync.dma_start(gt8_sb[:], gt8_d[:])

        wq_sb = consts.tile([128, NJ, C], BF)
        wk_sb = consts.tile([128, NJ, C], BF)
        wv_sb = consts.tile([128, NJ, C], F8)
        pw_sb = consts.tile([128, NJ, C], F8)
        for j in range(NJ):
            nc.sync.dma_start(wk_sb[:, j, :], wk_d[j * 128:(j + 1) * 128, :])
            nc.sync.dma_start(wq_sb[:, j, :], wq_d[j * 128:(j + 1) * 128, :])
        bq_sb = consts.tile([128, NJ], F32)
        bk_sb = consts.tile([128, NJ], F32)
        pb_sb = consts.tile([128, NJ], F32)
        for j in range(NJ):
            nc.sync.dma_start(bq_sb[:, j:j + 1], bq_d[j * 128:(j + 1) * 128])
            nc.sync.dma_start(bk_sb[:, j:j + 1], bk_d[j * 128:(j + 1) * 128])
        for j in range(NJ):
            nc.sync.dma_start(wv_sb[:, j, :], wv_d[j * 128:(j + 1) * 128, :])
        bv_bc = consts.tile([128, C], F32)      # v bias broadcast to all partitions
        nc.sync.dma_start(bv_bc[:], bass.AP(tensor=bv_d.tensor, offset=0, ap=[[0, 128], [1, C]]))
        sel4_sb = consts.tile([4, 2, 128], BF)
        nc.sync.dma_start(sel4_sb[:], sel4_d[:].rearrange("p (j m) -> p j m", j=2))
        for j in range(NJ):
            nc.sync.dma_start(pw_sb[:, j, :], pw_d[j * 128:(j + 1) * 128, :])
            nc.sync.dma_start(pb_sb[:, j:j + 1], pb_d[j * 128:(j + 1) * 128])

        eps_sb = consts.tile([128, 1], F32)
        nc.vector.memset(eps_sb[:], 1e-5)
        nb_sb = consts.tile([128, 1], F32)
        nc.vector.memset(nb_sb[:], EXP_BIAS)
        # zero the vT pad columns once (Ldweights loads the full 128-col slab);
        # on GpSimd so it doesn't block the DVE preamble chain
        nc.gpsimd.memset(vT_sb[:, :, :, CH + 1:128], 0.0)

        # ---- PE warm-up: ramp the tensor engine to full pstate during DMA wait
        wz = consts.tile([128, 512], BF)
        nc.vector.memset(wz[:], 0.0)
        wrd = consts.tile([128, 2], F32)
        for r in range(2):
            wps = acct(f"wps{r}")
            for i in range(8):
                nc.tensor.matmul(wps[:], wz[:, 0:128], wz[:], start=(i == 0), stop=(i == 7))
            nc.vector.tensor_copy(wrd[:, r:r + 1], wps[:, 0:1])

        # ---- GroupNorm: per-partition stats, group-reduce via tiny f32 matmuls ----
        stats_sb = consts.tile([128, 3, NJ], F32)  # rows: mean | var | mean^2
        for j in range(NJ):
            st6 = work.tile([128, 2, 6], F32, tag="st6")
            nc.vector.bn_stats(st6[:, 0, :], xt[:, j, 0:512])
            nc.vector.bn_stats(st6[:, 1, :], xt[:, j, 512:1024])
            nc.vector.bn_aggr(stats_sb[:, 0:2, j], st6[:])
        nc.vector.tensor_mul(stats_sb[:, 2, :], stats_sb[:, 0, :], stats_sb[:, 0, :])
        ps_st = acct("ps_st", (8, 3 * NJ))
        nc.tensor.matmul(ps_st[:], g8_sb[:], stats_sb[:].rearrange("p a b -> p (a b)"),
                         start=True, stop=True)
        st_g = work.tile([8, 3 * NJ], F32, tag="stg2")
        nc.vector.tensor_copy(st_g[:], ps_st[:])
        stv = st_g[:].rearrange("p (c j) -> p c j", c=3)
        bcin = work.tile([8, 8], F32, tag="bcin")
        vv = work.tile([8, NJ], F32, tag="vv")
        nc.vector.tensor_add(vv[:], stv[:, 1, :], stv[:, 2, :])
        m2 = work.tile([8, NJ], F32, tag="m2")
        nc.vector.tensor_mul(m2[:], stv[:, 0, :], stv[:, 0, :])
        nc.vector.tensor_sub(vv[:], vv[:], m2[:])
        nc.vector.tensor_copy(bcin[:, 0:4], stv[:, 0, :])
        nc.scalar.activation(vv[:], vv[:], AFT.Sqrt, bias=eps_sb[0:8, :], scale=1.0)
        # re-warm the Exp table right after Sqrt so the stream isn't table-delayed
        warm = work.tile([1, 1], BF, tag="warm", bufs=1)
        nc.scalar.activation(warm[:], eps_sb[0:1, :], AFT.Exp, bias=eps_sb[0:1, :], scale=1.0)
        nc.vector.reciprocal(bcin[:, 4:8], vv[:])
        ps_pp = acct("ps_pp", (128, 8))
        nc.tensor.matmul(ps_pp[:], gt8_sb[:], bcin[:], start=True, stop=True)
        ab = work.tile([128, 2 * NJ], F32, tag="ab")   # scale | shift per c-tile
        t4 = work.tile([128, NJ], F32, tag="t4")
        nc.vector.tensor_mul(ab[:, 0:NJ], ps_pp[:, 4:8], gns_sb[:])
        nc.vector.tensor_mul(t4[:], ps_pp[:, 0:4], ab[:, 0:NJ])
        nc.vector.tensor_sub(ab[:, NJ:2 * NJ], gnb_sb[:], t4[:])
        for j in range(NJ):
            nc.vector.tensor_scalar(xn[:, j, :], xt[:, j, :],
                                    ab[:, j:j + 1], ab[:, NJ + j:NJ + j + 1],
                                    op0=MULT, op1=ADD)

        # ---- filler chunk makers (all self-contained on the acc tag) ----
        def kq_chunk(m, part, n):
            # half of a k/q c-tile: [128,512] psum group over 4 j chunks
            def fn():
                w_sb = wk_sb if part == "k" else wq_sb
                dst = k_sb if part == "k" else q_sb
                b_sb = bk_sb if part == "k" else bq_sb
                ps = acct(f"ps{part}{m}{n}")
                for j in range(NJ):
                    nc.tensor.matmul(ps[:],
                                     w_sb[:, j, 128 * m:128 * (m + 1)],
                                     xn[:, j, 512 * n:512 * (n + 1)],
                                     start=(j == 0), stop=(j == NJ - 1))
                nc.vector.tensor_scalar(dst[:, m, 512 * n:512 * (n + 1)], ps[:],
                                        b_sb[:, m:m + 1], None, op0=ADD)
            return fn

        def v_single(tm):
            def fn():
                psv = acct(f"psv{tm}")
                for jp in range(2):
                    nc.tensor.matmul(psv[:],
                                     xn8[:, 2 * jp:2 * jp + 2, 128 * tm:128 * (tm + 1)],
                                     wv_sb[:, 2 * jp:2 * jp + 2, :],
                                     start=(jp == 0), stop=(jp == 1), perf_mode=DRM)
                nc.vector.tensor_add(vT_sb[:, tm, :, 0:CH],
                                     psv[:].rearrange("p (h c) -> p h c", h=H),
                                     bv_bc[:].rearrange("p (h c) -> p h c", h=H))
                nc.vector.memset(vT_sb[:, tm, :, CH:CH + 1], 1.0)
            return fn

        def xn8_chunk(j):
            def fn():
                nc.vector.tensor_copy(xn8[:, j, :], xn[:, j, :])
            return fn

        def proj_half(ms, n):
            # proj output halves for c-tiles ms at t-half n (fp8 DoubleRow)
            def fn():
                for m in ms:
                    psp = acct(f"psp{m}{n}")
                    for jp in range(2):
                        nc.tensor.matmul(psp[:],
                                         pw_sb[:, 2 * jp:2 * jp + 2, 128 * m:128 * (m + 1)],
                                         a_n[:, 2 * jp:2 * jp + 2, 512 * n:512 * (n + 1)],
                                         start=(jp == 0), stop=(jp == 1), perf_mode=DRM)
                    nc.vector.scalar_tensor_tensor(osb[:, m, 512 * n:512 * (n + 1)],
                                                   psp[:], pb_sb[:, m:m + 1],
                                                   xt[:, m, 512 * n:512 * (n + 1)],
                                                   op0=ADD, op1=ADD)
            return fn

        # ---- attention sweeps: 16 (pair-half u, s-chunk sm) units ----
        def attention_sweep(g, n, unit_order, fillers):
            ew = ewp.tile([128, NTM, 4, 512], F8, tag="ew", name=f"ew{g}{n}")
            sw = dict(g=g, n=n, ew=ew)
            sw["stg"] = work.tile([65, 4, 512], F32, tag="stg", name=f"stg{g}{n}")
            for idx, (u, sm) in enumerate(unit_order):
                p = 2 * g + u
                psw = pswt(f"psw{g}{n}{sm}{u}")
                for uu in range(2):
                    nc.tensor.matmul(psw[:, uu, :],
                                     k_sb[64 * uu:64 * (uu + 1), p, 128 * sm:128 * (sm + 1)],
                                     q_sb[64 * uu:64 * (uu + 1), p, 512 * n:512 * (n + 1)],
                                     start=True, stop=True, tile_position=(64 * uu, 0))
                nc.scalar.activation(
                    ew[:, sm, 2 * u:2 * u + 2, :].rearrange("p a b -> p (a b)"),
                    psw[:].rearrange("p a b -> p (a b)"),
                    AFT.Exp, bias=nb_sb[:], scale=EXP_SCALE)
                for f in fillers.get(idx, []):
                    f()
            return sw

        # AV burst for one head: fp8 DoubleRow over sm-pairs of the DONE ew tile
        def av_burst(sw, hh, smps, add_in):
            def fn():
                g, n, ew, stg = sw["g"], sw["n"], sw["ew"], sw["stg"]
                h = 4 * g + hh
                psa = acct(f"psa{g}{n}{hh}{smps[0]}")
                for i, smp in enumerate(smps):
                    nc.tensor.matmul(psa[:],
                                     vT_sb[:, 2 * smp:2 * smp + 2, h, :],
                                     ew[:, 2 * smp:2 * smp + 2, hh, :],
                                     start=(i == 0), stop=(i == len(smps) - 1),
                                     perf_mode=DRM)
                au = a_un[64 * (h % 2):64 * (h % 2) + 64, h // 2, 512 * n:512 * (n + 1)]
                if add_in:
                    nc.vector.tensor_add(stg[64:65, hh, :], psa[CH:CH + 1, :],
                                         stg[64:65, hh, :])
                    nc.vector.tensor_add(au, psa[0:CH, :], au)
                else:
                    nc.vector.tensor_copy(stg[64:65, hh, :], psa[CH:CH + 1, :])
                    nc.vector.tensor_copy(au, psa[0:CH, :])
            return fn

        def ep_rs(sw):
            rs4 = work.tile([4, 512], F32, tag="rs4")
            rc4 = work.tile([4, 512], F32, tag="rc4")
            sw["rc4"] = rc4

            def fn():
                nc.sync.dma_start(rs4[:], sw["stg"][64:65, :, :])
                nc.vector.reciprocal_approx_fast(rc4[:], rs4[:])
            return fn

        def ep_norm(sw):
            def fn():
                g, n = sw["g"], sw["n"]
                rcb = work.tile([4, 512], BF, tag="rcb")
                nc.vector.tensor_copy(rcb[:], sw["rc4"][:])
                for jj in range(2):
                    j = 2 * g + jj
                    psb = acct(f"psb{g}{n}{jj}")
                    nc.tensor.matmul(psb[:], sel4_sb[:, jj, :], rcb[:],
                                     start=True, stop=True)
                    nc.vector.tensor_mul(a_n[:, j, 512 * n:512 * (n + 1)],
                                         a_un[:, j, 512 * n:512 * (n + 1)], psb[:])
            return fn

        PMAJ = [(u, sm) for u in range(2) for sm in range(NTM)]
        SMAJ = [(u, sm) for sm in range(NTM) for u in range(2)]
        ALLSMP = list(range(NTM // 2))

        # preamble: k0/q0 only (first pair of group 0), n-split on acc
        kq_chunk(0, "k", 0)()
        kq_chunk(0, "k", 1)()
        kq_chunk(0, "q", 0)()
        kq_chunk(0, "q", 1)()

        sw00 = attention_sweep(0, 0, PMAJ, {
            0: [kq_chunk(1, "k", 0)], 1: [kq_chunk(1, "k", 1)],
            2: [kq_chunk(1, "q", 0)], 3: [kq_chunk(1, "q", 1)],
            4: [xn8_chunk(0), xn8_chunk(1)], 5: [xn8_chunk(2), xn8_chunk(3)],
            6: [v_single(0)], 7: [v_single(1)],
            8: [v_single(2)], 9: [v_single(3)], 10: [v_single(4)],
            11: [v_single(5)], 12: [v_single(6)], 13: [v_single(7)],
            14: [kq_chunk(2, "k", 0)], 15: [kq_chunk(2, "k", 1)]})
        sw01 = attention_sweep(0, 1, PMAJ, {
            0: [av_burst(sw00, 0, ALLSMP, False)],
            1: [av_burst(sw00, 1, ALLSMP, False)],
            2: [av_burst(sw00, 2, ALLSMP, False)],
            3: [av_burst(sw00, 3, ALLSMP, False)],
            4: [ep_rs(sw00)], 6: [ep_norm(sw00)],
            8: [kq_chunk(2, "q", 0)], 9: [kq_chunk(2, "q", 1)],
            10: [kq_chunk(3, "k", 0)], 11: [kq_chunk(3, "k", 1)],
            12: [kq_chunk(3, "q", 0)], 13: [kq_chunk(3, "q", 1)]})
        sw10 = attention_sweep(1, 0, PMAJ, {
            0: [av_burst(sw01, 0, ALLSMP, False)],
            1: [av_burst(sw01, 1, ALLSMP, False)],
            2: [av_burst(sw01, 2, ALLSMP, False)],
            3: [av_burst(sw01, 3, ALLSMP, False)],
            4: [ep_rs(sw01)], 6: [ep_norm(sw01)]})
        sw11 = attention_sweep(1, 1, SMAJ, {
            0: [av_burst(sw10, 0, ALLSMP, False)],
            1: [av_burst(sw10, 1, ALLSMP, False)],
            2: [av_burst(sw10, 2, ALLSMP, False)],
            3: [av_burst(sw10, 3, ALLSMP, False)],
            4: [ep_rs(sw10)], 6: [ep_norm(sw10)],
            7: [proj_half((0, 1), 0)], 8: [proj_half((2, 3), 0)]})

        # sw11's own partial bursts (exps for sm0..5 are done before its last
        # QK units execute, so these overlap the final exp windows)
        for hh in range(4):
            av_burst(sw11, hh, [0, 1, 2], False)()
        # tail: last sm-pair, reductions, normalize, proj n1, store
        for hh in range(4):
            av_burst(sw11, hh, [3], True)()
        ep_rs(sw11)()
        ep_norm(sw11)()
        proj_half((0, 1), 1)()
        proj_half((2, 3), 1)()
        for m in range(NJ):
            nc.sync.dma_start(out_d[128 * m:(m + 1) * 128, :], osb[:, m, :])

    nc.compile()
    return nc


_NC_CACHE = {}


def get_nc():
    if "nc" not in _NC_CACHE:
        _NC_CACHE["nc"] = build_graph()
    return _NC_CACHE["nc"]


def make_in_maps(x, norm_scale, norm_bias, qkv_w, qkv_b, proj_w, proj_b):
    x = np.asarray(x, dtype=np.float32)
    B = x.shape[0]
    qr = np.asarray(qkv_w, np.float32).reshape(H, 3, CH, C)
    wq = np.ascontiguousarray(qr[:, 0].reshape(C, C).T).astype(BF_NP)
    wk = np.ascontiguousarray(qr[:, 1].reshape(C, C).T).astype(BF_NP)
    wv = np.ascontiguousarray(qr[:, 2].reshape(C, C).T).astype(F8_NP)
    br = np.asarray(qkv_b, np.float32).reshape(H, 3, CH)
    bq = np.ascontiguousarray(br[:, 0].reshape(C))
    bk = np.ascontiguousarray(br[:, 1].reshape(C))
    bv = np.ascontiguousarray(br[:, 2].reshape(C))
    pw = np.ascontiguousarray(np.asarray(proj_w, np.float32).T).astype(F8_NP)
    pb = np.ascontiguousarray(np.asarray(proj_b, np.float32))
    g8 = np.zeros((128, 8), np.float32)
    g8[np.arange(128), np.arange(128) // 16] = 1.0 / 16.0
    gt8 = np.ascontiguousarray((g8 != 0).astype(np.float32).T)
    sel4 = np.zeros((4, 2, 128), np.float32)
    for jj in range(2):
        p = np.arange(128)
        sel4[2 * jj + p // 64, jj, p] = 1.0
    sel4 = np.ascontiguousarray(sel4.reshape(4, 256)).astype(BF_NP)
    shared = dict(wq=wq, wk=wk, wv=wv, pw=pw, bq=bq, bk=bk, bv=bv, pb=pb,
                  sel4=sel4,
                  gns=np.ascontiguousarray(np.asarray(norm_scale, np.float32)),
                  gnb=np.ascontiguousarray(np.asarray(norm_bias, np.float32)),
                  g8=g8, gt8=gt8)
    in_maps = []
    for i in range(B):
        m = dict(shared)
        m["x"] = np.ascontiguousarray(x[i].reshape(C, T)).astype(BF_NP)
        in_maps.append(m)
    return in_maps


def kernel(x, norm_scale, norm_bias, qkv_w, qkv_b, proj_w, proj_b):
    x = np.asarray(x, dtype=np.float32)
    B, Cc, Hh, Ww = x.shape
    nc = get_nc()
    in_maps = make_in_maps(x, norm_scale, norm_bias, qkv_w, qkv_b, proj_w, proj_b)
    res = run_bass_kernel_spmd(nc, in_maps, core_ids=list(range(B)))
    out = np.stack([res.results[i]["out"] for i in range(B)])
    return out.reshape(B, Cc, Hh, Ww).astype(np.float32)
